# revision 1
# baseline (speedup 1.0000x reference)
"""Trainium2 Bass kernel for fused self-attention (nn_Attention).

Reference computes (only q is used; k/v inputs are dead):
    qkv = q @ in_w.T + qkv_bias ; qp,kp,vp = split(qkv)
    per head: softmax(qp @ kp.T / sqrt(hd)) @ vp
    net = concat_heads @ out_w.T + out_b

Sharding: tensor-parallel over heads. 16 heads / 8 cores = 2 heads/core.
Each core projects q against its 2-head slice of in_w, runs attention for
its (2 batch x 2 head) pairs, and computes a partial output projection
against its 128 columns of out_w. Host sums the 8 partials.

On-device layouts (matmul operands fp16, accumulation fp32 in PSUM):
  qT      [1024(d), 4096(b*2048+s)]  q transposed (host prep)
  qk_sb   [128(o), 2(Q/K), 4096(s)]  projected Q,K transposed; partition =
                                     head o-dims (h0: 0-63, h1: 64-127)
  v_sb    [128(t), b, tt, 130]       V in [token, dim] layout: h0 dims 0-63,
                                     ones col 64, h1 dims 65-128, ones col
                                     129 -> each head's PV lhsT [t, 65] slice
                                     is contiguous; the ones column makes the
                                     PV matmul also produce the softmax
                                     denominator (row 64 of pv)
  scoresT [128(t), 2(head), 512] PSUM, double-buffered; h0/h1 matmuls run
          concurrently in distinct PE row groups; one exp op per tile
  pv      [65, 512] per (head, chunk): rows 0-63 out.T, row 64 = denom
  normalize: DVE reciprocal + GpSimd partition_broadcast + DVE multiply
  proj    partial[o, s]: per (ot, s-half): 2 matmuls -> staged copy ->
          one [128, 1024] store

Scheduling: attention is an ACT(exp)-bound software pipeline (scores ->
exp one tile ahead of pv). All other work -- the rest of the b=0 QKV
projection, the entire b=1 QKV projection, and both output projections --
is split into ~1us parts and woven into specific (chunk, tt) emission
slots of the attention loops, ordered to respect streaming deadlines
(K units feed score t-tiles, V units feed pv t-tiles). Dummy matmuls
warm the PE clock gate (HAM) during the initial q-load wait, and a
dummy exp forces the ACT table load before DMAs occupy the queues.
PSUM budget: 2x2 score banks + 4 shared pv/weave banks = 8.
"""

import sys

for p in ("/opt/trn_rl_repo", "/root/.axon_site/_ro/trn_rl_repo"):
    if p not in sys.path:
        sys.path.append(p)

import numpy as np

B, S, D, H = 2, 2048, 1024, 16
BS = B * S  # 4096
HD = 64  # head dim
NCORES = 8
HPC = H // NCORES  # 2 heads per core -> 128 o-dims per core

_COMPILED = {}


def _build():
    import concourse.bass as bass  # noqa: F401
    import concourse.mybir as mybir
    import concourse.tile as tile
    from concourse import bacc
    from concourse.masks import make_identity

    f16 = mybir.dt.float16
    f32 = mybir.dt.float32
    AF = mybir.ActivationFunctionType

    nc = bacc.Bacc("TRN2", target_bir_lowering=False, debug=False,
                   num_devices=NCORES)

    qT_d = nc.declare_dram_parameter("qT", [D, BS], f16, isOutput=False)
    wqk_d = nc.declare_dram_parameter("wqk", [D, 256], f16, isOutput=False)
    wv_d = nc.declare_dram_parameter("wv", [D, 128], f16, isOutput=False)
    w2_d = nc.declare_dram_parameter("w2", [128, D], f16, isOutput=False)
    qkb_d = nc.declare_dram_parameter("qkb", [1, 256], f16, isOutput=False)
    vb_d = nc.declare_dram_parameter("vb", [1, 128], f16, isOutput=False)
    out_d = nc.declare_dram_parameter("partial", [D, BS], f16, isOutput=True)

    with tile.TileContext(nc) as tc:
        with (
            tc.tile_pool(name="persist", bufs=1) as persist,
            tc.tile_pool(name="exp", bufs=6) as exp_pool,
            tc.tile_pool(name="outT", bufs=2) as outT_pool,
            tc.tile_pool(name="recip", bufs=4) as recip_pool,
            tc.tile_pool(name="rep", bufs=4) as rep_pool,
            tc.tile_pool(name="stage", bufs=3) as stage_pool,
        ):
            # ---- resident SBUF tensors ----
            q_sb = persist.tile([128, 8, BS], f16)     # 64KB/part
            wqk_sb = persist.tile([128, 8, 256], f16)
            wv_sb = persist.tile([128, 8, 128], f16)
            w2_sb = persist.tile([128, D], f16)
            qkb_sb = persist.tile([1, 256], f16)
            vb_sb = persist.tile([1, 128], f16)
            ones_sb = persist.tile([1, 512], f16)
            qk_sb = persist.tile([128, 2, BS], f16)    # 16KB/part
            # V layout per (b, t-tile): cols 0-63 h0 dims, col 64 ones,
            # cols 65-128 h1 dims, col 129 ones -> each head's PV lhsT
            # [t, 65] slice is contiguous with its denominator in row 64
            v_sb = persist.tile([128, B, 16, 130], f16)
            ident_sb = persist.tile([128, 128], f16)
            warm_sb = persist.tile([1, 8], f32)
            nc.vector.memset(ones_sb[:, :], 1.0)
            make_identity(nc, ident_sb[:, :])
            # force the exp ACT-table load NOW, before big DMAs occupy the
            # queues -- otherwise the implicit table load lands behind them
            # and gates the first real exp by ~20us
            nc.vector.memset(warm_sb[:, :], 0.0)
            nc.scalar.activation(warm_sb[:, :], warm_sb[:, :], AF.Exp)

            # loads ordered by first use: weights for chunk-0 units first,
            # then q chunks in streaming order
            qT_t = qT_d.rearrange("(n p) m -> p n m", p=128)
            nc.sync.dma_start(wqk_sb[:, :, :],
                              wqk_d.rearrange("(n p) m -> p n m", p=128))
            nc.sync.dma_start(wv_sb[:, :, :],
                              wv_d.rearrange("(n p) m -> p n m", p=128))
            nc.sync.dma_start(qkb_sb[:, :], qkb_d[:, :])
            nc.sync.dma_start(vb_sb[:, :], vb_d[:, :])
            nc.sync.dma_start(w2_sb[:, :], w2_d[:, :])
            for scc in range(8):  # q arrives per 512-chunk: units stream
                nc.sync.dma_start(
                    q_sb[:, :, scc * 512:(scc + 1) * 512],
                    qT_t[:, :, scc * 512:(scc + 1) * 512],
                )

            # ---- work-unit emitters -------------------------------------
            # Each returns a closure that emits one psum-group of work using
            # the given pool. Units are either run solid (phase 1 for b=0) or
            # woven one-at-a-time into the attention loop's PE slack.
            def qkv_unit(pool, b, m, scc, tag, nm):
                """One projection psum-group: m=0 Q, m=1 K (-> qk_sb) or
                m=2 V (-> vT staging -> PE transpose into v_sb). Split into
                two ~1us parts so woven units never delay the exp-feeding
                score matmuls by more than ~1us on the in-order PE stream."""
                s0 = b * 2048 + scc * 512
                ref = {}

                def mm_half(lo):
                    for dk in range(lo, lo + 4):
                        w = (wqk_sb[:, dk, m * 128:(m + 1) * 128] if m < 2
                             else wv_sb[:, dk, :])
                        nc.tensor.matmul(
                            ref["ps"][:, :],
                            w,
                            q_sb[:, dk, s0:s0 + 512],
                            start=(dk == 0), stop=False,
                        )

                def part_a():
                    ref["ps"] = pool.tile([128, 512], f32, tag=tag, name=nm)
                    mm_half(0)

                def part_b():
                    ps = ref["ps"]
                    mm_half(4)
                    brow = (qkb_sb[0:1, m * 128:(m + 1) * 128] if m < 2
                            else vb_sb[0:1, :])
                    nc.tensor.matmul(  # += bias_row.T @ ones
                        ps[:, :],
                        brow,
                        ones_sb[0:1, :],
                        start=False, stop=True,
                    )
                    if m < 2:
                        nc.vector.tensor_copy(qk_sb[:, m, s0:s0 + 512], ps[:, :])
                    else:
                        vt = vt_pool.tile([128, 512], f16, tag="vt",
                                          name=f"vt{nm}")
                        nc.vector.tensor_copy(vt[:, :], ps[:, :])
                        for sub in range(4):
                            st = scc * 4 + sub
                            tr = pool.tile([128, 128], f16, tag=tag,
                                           name=f"tr{nm}_{sub}")
                            nc.tensor.transpose(
                                tr[:, :],
                                vt[:, sub * 128:(sub + 1) * 128],
                                ident_sb[:, :])
                            nc.vector.tensor_copy(v_sb[:, b, st, 0:64],
                                                  tr[:, 0:64])
                            nc.vector.tensor_copy(v_sb[:, b, st, 65:129],
                                                  tr[:, 64:128])
                            nc.vector.memset(v_sb[:, b, st, 64:65], 1.0)
                            nc.vector.memset(v_sb[:, b, st, 129:130], 1.0)
                return [part_a, part_b]

            def v_small_unit(pool, b, st, tag, nm):
                """Direct V projection for one t-tile (slower on PE but
                self-contained -> fast availability for streaming deadlines)."""
                def emit():
                    t0 = b * 2048 + st * 128
                    ps = pool.tile([128, 128], f32, tag=tag, name=nm)
                    for dk in range(8):
                        nc.tensor.matmul(
                            ps[:, :],
                            q_sb[:, dk, t0:t0 + 128],
                            wv_sb[:, dk, :],
                            start=(dk == 0), stop=False,
                        )
                    nc.tensor.matmul(
                        ps[:, :],
                        ones_sb[0:1, 0:128],
                        vb_sb[0:1, :],
                        start=False, stop=True,
                    )
                    nc.vector.tensor_copy(v_sb[:, b, st, 0:64], ps[:, 0:64])
                    nc.vector.tensor_copy(v_sb[:, b, st, 65:129],
                                          ps[:, 64:128])
                    nc.vector.memset(v_sb[:, b, st, 64:65], 1.0)
                    nc.vector.memset(v_sb[:, b, st, 129:130], 1.0)
                return [emit]

            def proj_unit(pool, b, ot, outT_sb, nm):
                def half(lo):
                    # self-contained half: 2 matmuls -> staged copy -> one
                    # [128, 1024] store; no state spans the two parts
                    stage = stage_pool.tile([128, 1024], f16, tag="st",
                                            name=f"st{nm}_{lo}")
                    for j, sc in enumerate((lo, lo + 1)):
                        ps = pool.tile([128, 512], f32, tag="pv",
                                       name=f"pj{nm}_{sc}")
                        nc.tensor.matmul(
                            ps[:, :],
                            w2_sb[:, ot * 128:(ot + 1) * 128],
                            outT_sb[:, sc, :],
                            start=True, stop=True,
                        )
                        nc.vector.tensor_copy(
                            stage[:, j * 512:(j + 1) * 512], ps[:, :])
                    nc.sync.dma_start(
                        out_d[ot * 128:(ot + 1) * 128,
                              b * 2048 + lo * 512:b * 2048 + (lo + 2) * 512],
                        stage[:, :],
                    )
                return [lambda: half(0), lambda: half(2)]

            vt_cm = tc.tile_pool(name="vt", bufs=3)
            vt_pool = vt_cm.__enter__()

            # ---- phase 1: QKV projection for b=0 chunks 0-1 (solid);
            # the rest streams into the attention loop's PE slack ----
            with tc.tile_pool(name="qkv0", bufs=4, space="PSUM") as qkv0_pool:
                # dummy matmuls fill the q-load wait: they warm the PE clock
                # gate (HAM) so the real projection runs at full rate
                wps = qkv0_pool.tile([128, 128], f32, tag="warm", name="wps")
                for i in range(80):
                    nc.tensor.matmul(wps[:, :], ident_sb[:, :], ident_sb[:, :],
                                     start=True, stop=True)
                for scc in range(2):
                    for m in range(3):
                        for part in qkv_unit(qkv0_pool, 0, m, scc, "p0",
                                             f"u0{m}{scc}"):
                            part()

            # ---- attention per b, with deferred work woven in ----
            with tc.tile_pool(name="scps", bufs=2, space="PSUM") as scps_pool, \
                 tc.tile_pool(name="pvps", bufs=4, space="PSUM") as pvps_pool:
                outT_tiles = {}
                tail_parts = []
                for b in range(B):
                    outT_sb = outT_pool.tile([128, 4, 512], f16, tag="outT",
                                             name=f"outT{b}")
                    outT_tiles[b] = outT_sb
                    # (chunk, tt) -> work units woven at that emission slot.
                    # Emission position is a hard dependency deadline: a unit
                    # feeding scores(tt)/pv(tt) must be emitted before them.
                    sched = {}

                    def assign(slots, parts):
                        assert len(slots) >= len(parts), (len(slots), len(parts))
                        for s, p in zip(slots, parts):
                            sched.setdefault(s, []).append(p)

                    if b == 0:
                        # rest of qkv(b0) ahead of its streaming deadlines
                        # (K unit scc feeds score t-tiles 4scc.., small V
                        # units feed pv t-tiles), then all of qkv(b1)
                        assign([(0, 1), (0, 2)],
                               qkv_unit(pvps_pool, 0, 1, 2, "pv", "u012"))
                        for i, st in enumerate((8, 9, 10, 11)):
                            assign([(0, 3 + i)],
                                   v_small_unit(pvps_pool, 0, st, "pv", f"vs{st}"))
                        assign([(0, 7), (0, 8)],
                               qkv_unit(pvps_pool, 0, 1, 3, "pv", "u013"))
                        for i, st in enumerate((12, 13, 14, 15)):
                            assign([(0, 9 + i)],
                                   v_small_unit(pvps_pool, 0, st, "pv", f"vs{st}"))
                        assign([(0, 13), (0, 14)],
                               qkv_unit(pvps_pool, 0, 0, 2, "pv", "u002"))
                        assign([(0, 15), (1, 1)],
                               qkv_unit(pvps_pool, 0, 0, 3, "pv", "u003"))
                        b1p = []
                        for scc in range(4):
                            for m in range(3):
                                b1p += qkv_unit(pvps_pool, 1, m, scc, "pv",
                                                f"u1{m}{scc}")
                        slots = ([(1, t) for t in range(2, 16)]
                                 + [(2, t) for t in range(1, 16, 2)]
                                 + [(3, t) for t in range(1, 16, 2)])
                        assert len(slots) >= len(b1p)
                        assign(slots, b1p)
                    else:  # projection of b=0 hides inside attention(b=1);
                        # proj(b=1) first halves ride chunks 2-3 (their outT
                        # chunks 0-1 are ready), second halves run in the tail
                        pp = []
                        for i in range(8):
                            pp += proj_unit(pvps_pool, 0, i, outT_tiles[0],
                                            f"0_{i}")
                        assign([(0, t) for t in range(1, 16, 2)]
                               + [(1, t) for t in range(1, 16, 2)], pp)
                        p1 = [proj_unit(pvps_pool, 1, i, outT_sb, f"1_{i}")
                              for i in range(8)]
                        assign([(2, t) for t in range(1, 16, 2)],
                               [u[0] for u in p1])
                        tail_parts.extend(u[1] for u in p1)
                    for ch in range(4):  # 512-wide s-chunks
                        s0 = b * 2048 + ch * 512
                        pv = [pvps_pool.tile([65, 512], f32, tag="pv",
                                             name=f"pv{b}_{ch}_{h}")
                              for h in range(HPC)]
                        prev_e = None
                        for tt in range(16):
                            t0 = b * 2048 + tt * 128
                            sc_ps = scps_pool.tile([128, 2, 512], f32, tag="sc",
                                                   name=f"sc{b}_{ch}_{tt}")
                            # h0/h1 back-to-back -> concurrent PE row groups
                            for h in range(HPC):
                                lo, hi = h * 64, (h + 1) * 64
                                nc.tensor.matmul(
                                    sc_ps[:, h, :],
                                    qk_sb[lo:hi, 1, t0:t0 + 128],
                                    qk_sb[lo:hi, 0, s0:s0 + 512],
                                    start=True, stop=True,
                                )
                            for u in sched.get((ch, tt), ()):
                                u()
                            # pv runs one iteration behind so exp(tt) overlaps
                            # pv(tt-1) and scores(tt+1) on PE
                            if prev_e is not None:
                                pe, ptt = prev_e
                                for h in range(HPC):
                                    nc.tensor.matmul(
                                        pv[h][:, :],
                                        v_sb[:, b, ptt, 65 * h:65 * h + 65],
                                        pe[:, h, :],
                                        start=(ptt == 0), stop=False,
                                    )
                            e = exp_pool.tile([128, 2, 512], f16, tag="exp",
                                              name=f"e{b}_{ch}_{tt}")
                            nc.scalar.activation(e[:, :, :], sc_ps[:, :, :],
                                                 AF.Exp, scale=0.125)
                            prev_e = (e, tt)
                        pe, ptt = prev_e
                        for h in range(HPC):
                            nc.tensor.matmul(
                                pv[h][:, :],
                                v_sb[:, b, ptt, 65 * h:65 * h + 65],
                                pe[:, h, :],
                                start=False, stop=True,
                            )
                        # normalize: denom row (64 for h0, 0 for h1) ->
                        # reciprocal -> partition broadcast -> multiply
                        for h in range(HPC):
                            recip = recip_pool.tile([1, 512], f32, tag="rc",
                                                    name=f"rc{b}{ch}{h}")
                            nc.vector.reciprocal(recip[:, :], pv[h][64:65, :])
                            rep = rep_pool.tile([64, 512], f32, tag="rep",
                                                name=f"rp{b}{ch}{h}")
                            nc.gpsimd.partition_broadcast(rep[:, :], recip[:, :])
                            nc.vector.tensor_mul(
                                outT_sb[h * 64:(h + 1) * 64, ch, :],
                                pv[h][0:64, :],
                                rep[:, :],
                            )
                for p in tail_parts:
                    p()
            vt_cm.__exit__(None, None, None)
    nc.compile()
    return nc


def _get_nc():
    if "nc" not in _COMPILED:
        _COMPILED["nc"] = _build()
    return _COMPILED["nc"]


def _prep_inputs(q, in_w, qkv_bias):
    f16 = np.float16
    qT = np.ascontiguousarray(q.transpose(2, 0, 1).reshape(D, BS)).astype(f16)
    maps = []
    for c in range(NCORES):
        r = slice(128 * c, 128 * (c + 1))
        wq, wk, wv = in_w[0:D][r], in_w[D:2 * D][r], in_w[2 * D:3 * D][r]
        maps.append({
            "qT": qT,
            "wqk": np.ascontiguousarray(np.concatenate([wq, wk], 0).T).astype(f16),
            "wv": np.ascontiguousarray(wv.T).astype(f16),
            "w2": None,  # filled with out_w slice
            "qkb": np.ascontiguousarray(
                np.concatenate([qkv_bias[0:D][r], qkv_bias[D:2 * D][r]])[None, :]
            ).astype(f16),
            "vb": np.ascontiguousarray(
                qkv_bias[2 * D:3 * D][r][None, :]
            ).astype(f16),
        })
    return maps


def kernel(q, k, v, in_w, qkv_bias, out_w, out_b, _trace=False):
    from concourse.bass_utils import run_bass_kernel_spmd

    q = np.asarray(q, dtype=np.float32)
    in_w = np.asarray(in_w, dtype=np.float32)
    qkv_bias = np.asarray(qkv_bias, dtype=np.float32)
    out_w = np.asarray(out_w, dtype=np.float32)
    out_b = np.asarray(out_b, dtype=np.float32)

    nc = _get_nc()
    in_maps = _prep_inputs(q, in_w, qkv_bias)
    for c in range(NCORES):
        r = slice(128 * c, 128 * (c + 1))
        in_maps[c]["w2"] = np.ascontiguousarray(out_w[:, r].T).astype(np.float16)

    res = run_bass_kernel_spmd(
        nc, in_maps, core_ids=list(range(NCORES)), trace=_trace,
    )
    total = np.zeros((D, BS), dtype=np.float32)
    for c in range(NCORES):
        total += res.results[c]["partial"].astype(np.float32)
    net = total.T + out_b[None, :]
    out = net.reshape(B, S, D).astype(np.float32)
    if _trace:
        return out, res
    return out



# revision 16
# speedup vs baseline: 1.2608x; 1.2608x over previous
"""Trainium2 Bass kernel for fused self-attention (nn_Attention), v3.

Reference (k/v inputs dead; only q used):
    qkv = q @ in_w.T + qkv_bias ; qp,kp,vp = split(qkv)
    per head: softmax(qp @ kp.T / 8) @ vp ;  net = concat @ out_w.T + out_b

Sharding: tensor-parallel over heads, 2 heads/core. Core c computes its
2-head slice end-to-end plus a partial output projection against its 128
columns of out_w; host sums the 8 partials (plus the V-bias term, which
commutes through softmax, added host-side as vb @ out_w.T).

Cost-model facts driving the design: matmul = out_free x pe_cycle x cpr
(fp16 cpr=1; fp8e4+DoubleRow cpr=0.5 and contracts two k-tiles per call);
ACT exp = 0.833ns/elem/lane -> the ~133us pacing floor; gpsimd cannot
read PSUM. Plain fp8 operands fail the 2e-2 gate (~5e-2 measured), so
fp8 appears only as hi/lo splits (x ~ hi8 + lo8, ~0.3% error):

  - QKV proj: 3-pass hi/lo fp8 DR (qh*wh + ql*wh + qh*wl; lo*lo dropped).
    Host preps q and 16*w hi/lo pairs; descale+bias fold into the
    PSUM->SBUF copies (Q also folds the 1/8 score scale, K/V fold 1/16).
  - scores, per (h, t-tile): DR pass kh*qh + kl*qh (slots [kh,kl] x
    [qh,qh-dup]) + plain-f8 pass kh*ql; ql*kl dropped. 320ns/tile vs
    fp16's 426.
  - exp: ACT, psum [128,2(h),512] -> e2 f16 (a tunable subset of tiles
    runs on DVE via a Schraudolph bit-trick into int16-viewed f16).
  - PV fp16, s-on-partitions: out[s-tile,65] per (h,st), accumulated over
    t-tiles; v_sb [t,65] per head with ones col 64 -> accum col 64 = the
    softmax denominator, already per-partition. The 8 accumulators are
    sub-bank packed into two persistent psum tiles ([128,388] f32: 4
    accums + two [128,128]-f16 transpose slots each). Only the first
    matmul per bank/chunk starts and the last stops: zero-region marking
    is bank-granular, later first-writes land on pending-zero bytes =
    fresh write (validated on HW).
  - normalize: ONE DVE copy per h (psum [128,260] -> pvc f16) is the only
    pv reader that gates the next chunk's accumulation; then DVE
    reciprocal [128,1] per (h,st) + gpsimd multiply (SBUF-only) ->
    norm_sb [s, 128(h0|h1)] f16; PE-transpose into the pv tile tails
    (start=False: those bytes are pending-zero from the current chunk's
    group start); DVE copy -> outT_sb [d, 512] f16.
  - out-proj fp16: lhsT=outT[:,st*128:], rhs=w2 [128,512] -> one psum
    bank; stage copy f16; DMA per s-tile.
  - PSUM: scores 2x2 banks + pv 2x1 + weave 2x1 = 8.
  - Schedule: the 128 exp tiles pace everything; projection/normalize/
    out-proj units are woven into per-tile slots with static deadlines
    (K/Q/V stream ahead of first use, pv lags exp by 3-4 tiles, chunk
    ch's tail rides chunk ch+1).
"""

import sys

for p in ("/opt/trn_rl_repo", "/root/.axon_site/_ro/trn_rl_repo"):
    if p not in sys.path:
        sys.path.append(p)

import numpy as np

B, S, D, H = 2, 2048, 1024, 16
BS = B * S
NCORES = 8
NTILE = 16

_COMPILED = {}

# in-chunk slots whose exp runs on DVE (Schraudolph) instead of ACT
DVE_EXP_TT = ()
SCHRAU_A = 1024.0 / np.log(2.0)
SCHRAU_B = 15.0 * 1024.0 + 0.5 - 29.0


def _build():
    import concourse.bass as bass  # noqa: F401
    import concourse.mybir as mybir
    import concourse.tile as tile
    from concourse import bacc
    from concourse.masks import make_identity

    f8 = mybir.dt.float8e4
    f16 = mybir.dt.float16
    f32 = mybir.dt.float32
    i16 = mybir.dt.int16
    AF = mybir.ActivationFunctionType
    DR = mybir.MatmulPerfMode.DoubleRow
    MUL = mybir.AluOpType.mult
    ADD = mybir.AluOpType.add
    SUB = mybir.AluOpType.subtract

    nc = bacc.Bacc("TRN2", target_bir_lowering=False, debug=False,
                   num_devices=NCORES)

    qh_d = nc.declare_dram_parameter("qh", [128, 8, BS], f8, isOutput=False)
    ql_d = nc.declare_dram_parameter("ql", [128, 8, BS], f8, isOutput=False)
    wqkh_d = nc.declare_dram_parameter("wqkh", [128, 8, 256], f8, isOutput=False)
    wqkl_d = nc.declare_dram_parameter("wqkl", [128, 8, 256], f8, isOutput=False)
    wvh_d = nc.declare_dram_parameter("wvh", [128, 8, 128], f8, isOutput=False)
    wvl_d = nc.declare_dram_parameter("wvl", [128, 8, 128], f8, isOutput=False)
    w2_d = nc.declare_dram_parameter("w2", [128, 1024], f16, isOutput=False)
    qkb_d = nc.declare_dram_parameter("qkb", [128, 2], f32, isOutput=False)
    out_d = nc.declare_dram_parameter("partialT", [BS, 1024], f16, isOutput=True)

    with tile.TileContext(nc) as tc:
        with (
            tc.tile_pool(name="persist", bufs=1) as persist,
            tc.tile_pool(name="e2", bufs=6) as e2_pool,
            tc.tile_pool(name="pvc", bufs=2) as pvc_pool,
            tc.tile_pool(name="r", bufs=8) as r_pool,
            tc.tile_pool(name="norm", bufs=4) as norm_pool,
            tc.tile_pool(name="outT", bufs=2) as outT_pool,
            tc.tile_pool(name="stage", bufs=3) as stage_pool,
            tc.tile_pool(name="scps", bufs=2, space="PSUM") as scps_pool,
            tc.tile_pool(name="pvps", bufs=2, space="PSUM") as pvps_pool,
            tc.tile_pool(name="wvps", bufs=2, space="PSUM") as wvps_pool,
        ):
            qh_sb = persist.tile([128, 8, BS], f8)      # 32KB/part
            ql_sb = persist.tile([128, 8, BS], f8)      # 32KB/part
            wqkh_sb = persist.tile([128, 8, 256], f8)
            wqkl_sb = persist.tile([128, 8, 256], f8)
            wvh_sb = persist.tile([128, 8, 128], f8)
            wvl_sb = persist.tile([128, 8, 128], f8)
            w2_sb = persist.tile([128, 1024], f16)
            qkb_sb = persist.tile([128, 2], f32)
            # projected Q/8 (m=0) and K (m=1), partitions h*64+d
            qk_sb = persist.tile([128, 2, BS], f16)     # 16KB/part
            v_sb = persist.tile([128, B, 2, NTILE, 65], f16)  # 16.3KB/part
            ident_sb = persist.tile([128, 128], f16)
            dummy_sb = persist.tile([128, 128], f8)
            warm_sb = persist.tile([1, 8], f32)

            # persistent pv psum tiles (one bank each): 4 accums [128,65]
            pv_ps = [pvps_pool.tile([128, 260], f32, tag="pv", name=f"pv{h}")
                     for h in range(2)]

            # ---- preamble ----
            nc.vector.memset(warm_sb[:, :], 0.0)
            nc.scalar.activation(warm_sb[:, :], warm_sb[:, :], AF.Exp)
            nc.vector.memset(dummy_sb[:, :], 0.0)
            make_identity(nc, ident_sb[:, :])
            for b in range(B):
                nc.gpsimd.memset(v_sb[:, b, :, :, 64:65], 1.0)

            nc.sync.dma_start(wqkh_sb[:, :, :], wqkh_d[:, :, :])
            nc.sync.dma_start(wqkl_sb[:, :, :], wqkl_d[:, :, :])
            nc.sync.dma_start(qkb_sb[:, :], qkb_d[:, :])
            nc.sync.dma_start(wvh_sb[:, :, :], wvh_d[:, :, :])
            nc.sync.dma_start(wvl_sb[:, :, :], wvl_d[:, :, :])
            nc.sync.dma_start(w2_sb[:, :], w2_d[:, :])
            for scc in range(8):
                nc.sync.dma_start(qh_sb[:, :, scc * 512:(scc + 1) * 512],
                                  qh_d[:, :, scc * 512:(scc + 1) * 512])
                nc.sync.dma_start(ql_sb[:, :, scc * 512:(scc + 1) * 512],
                                  ql_d[:, :, scc * 512:(scc + 1) * 512])

            wps = wvps_pool.tile([128, 512], f32, tag="wv", name="warm")
            for i in range(90):
                nc.tensor.matmul(wps[:, 0:128], dummy_sb[:, :], dummy_sb[:, :],
                                 start=True, stop=True)

            # ---- work units ----
            def qk_unit(b, m, hc):
                """Q (m=0) / K (m=1) projection, one 256-col half-chunk:
                3 hi/lo DR passes + descale/bias copies into q8/k8 slots."""
                def emit():
                    s0 = b * 2048 + hc * 256
                    ps = wvps_pool.tile([128, 512], f32, tag="wv",
                                        name=f"qk{b}{m}{hc}")
                    ops = [(qh_sb, wqkh_sb), (ql_sb, wqkh_sb), (qh_sb, wqkl_sb)]
                    for pi, (qs, ws) in enumerate(ops):
                        for j in range(4):
                            nc.tensor.matmul(
                                ps[:, 0:256],
                                ws[:, 2 * j:2 * j + 2, m * 128:(m + 1) * 128],
                                qs[:, 2 * j:2 * j + 2, s0:s0 + 256],
                                start=(pi == 0 and j == 0),
                                stop=(pi == 2 and j == 3), perf_mode=DR)
                    dsc = (1.0 / 128.0) if m == 0 else (1.0 / 16.0)
                    nc.vector.tensor_scalar(
                        qk_sb[:, m, s0:s0 + 256], ps[:, 0:256],
                        dsc, qkb_sb[:, m:m + 1], MUL, ADD)
                return emit

            def v_unit(b, t):
                """V projection for one t-tile, direct [t, dv]: 3 hi/lo DR
                passes + one strided descale copy into both head slots."""
                def emit():
                    t0 = b * 2048 + t * 128
                    ps = wvps_pool.tile([128, 512], f32, tag="wv",
                                        name=f"v{b}{t}")
                    ops = [(qh_sb, wvh_sb), (ql_sb, wvh_sb), (qh_sb, wvl_sb)]
                    for pi, (qs, ws) in enumerate(ops):
                        for j in range(4):
                            nc.tensor.matmul(
                                ps[:, 0:128],
                                qs[:, 2 * j:2 * j + 2, t0:t0 + 128],
                                ws[:, 2 * j:2 * j + 2, :],
                                start=(pi == 0 and j == 0),
                                stop=(pi == 2 and j == 3), perf_mode=DR)
                    nc.vector.tensor_scalar(
                        v_sb[:, b, :, t, 0:64], ps[:, 0:128],
                        1.0 / 16.0, None, MUL)
                return emit

            pvc_tiles = {}
            norm_tiles = {}
            outT_tiles = {}
            stage_tiles = {}

            def pv_read(b, ch):
                """The only pv psum readers: one [128,260] f32->f16 copy per
                head. Everything downstream works from SBUF."""
                def emit():
                    for h in range(2):
                        t = pvc_pool.tile([128, 4, 65], f16, tag="pvc",
                                          name=f"pvc{b}{ch}{h}")
                        pvc_tiles[(b, ch, h)] = t
                        nc.vector.tensor_copy(t[:, :, :], pv_ps[h][:, 0:260])
                return emit

            def normalize(b, ch, st):
                def emit():
                    nm = norm_pool.tile([128, 128], f16, tag="nm",
                                        name=f"nm{b}{ch}{st}")
                    norm_tiles[(b, ch, st)] = nm
                    for h in range(2):
                        pvc = pvc_tiles[(b, ch, h)]
                        r = r_pool.tile([128, 1], f32, tag="r",
                                        name=f"r{b}{ch}{h}{st}")
                        nc.vector.reciprocal(r[:, :], pvc[:, st, 64:65])
                        nc.gpsimd.tensor_scalar(
                            nm[:, h * 64:(h + 1) * 64],
                            pvc[:, st, 0:64], r[:, 0:1], None, MUL)
                return emit

            def transpose_unit(b, ch, st):
                """norm [s,128] -> [d,s] via PE transpose (own wvps group),
                then DVE copy into outT_sb."""
                def emit():
                    ps = wvps_pool.tile([128, 512], f32, tag="wv",
                                        name=f"tr{b}{ch}{st}")
                    tr = ps[:, 0:64].bitcast(f16)
                    nc.tensor.transpose(tr[:, :], norm_tiles[(b, ch, st)][:, :],
                                        ident_sb[:, :])
                    nc.vector.tensor_copy(
                        outT_tiles[(b, ch)][:, st * 128:(st + 1) * 128],
                        tr[:, :])
                return emit

            def outT_alloc(b, ch):
                def emit():
                    outT_tiles[(b, ch)] = outT_pool.tile(
                        [128, 512], f16, tag="oT", name=f"oT{b}{ch}")
                return emit

            def outproj_unit(b, ch, st, op):
                def emit():
                    ps = wvps_pool.tile([128, 512], f32, tag="wv",
                                        name=f"op{b}{ch}{st}{op}")
                    nc.tensor.matmul(
                        ps[:, :],
                        outT_tiles[(b, ch)][:, st * 128:(st + 1) * 128],
                        w2_sb[:, op * 512:(op + 1) * 512],
                        start=True, stop=True)
                    stg = stage_tiles[(b, ch, st)]
                    nc.vector.tensor_copy(stg[:, op * 512:(op + 1) * 512],
                                          ps[:, :])
                return emit

            def stage_alloc(b, ch, st):
                def emit():
                    stage_tiles[(b, ch, st)] = stage_pool.tile(
                        [128, 1024], f16, tag="st", name=f"st{b}{ch}{st}")
                return emit

            def dma_out(b, ch, st):
                def emit():
                    t0 = b * 2048 + ch * 512 + st * 128
                    nc.sync.dma_start(out_d[t0:t0 + 128, :],
                                      stage_tiles[(b, ch, st)][:, :])
                return emit

            # ---- static weave schedule ----
            NSLOT = B * 4 * NTILE
            sched = {i: [] for i in range(NSLOT + 1)}

            def put(g, *fns):
                sched[max(0, min(g, NSLOT))].extend(fns)

            pre_units = [qk_unit(0, 1, 0), qk_unit(0, 0, 0), qk_unit(0, 0, 1),
                         v_unit(0, 0), v_unit(0, 1)]
            for hc in range(1, 8):                       # b0 K: by slot 2hc
                put(2 * hc - 2, qk_unit(0, 1, hc))
            for hc in range(2, 8):                       # b0 Q: by 16*(hc//2)
                put(16 * (hc // 2) - 8 + 2 * (hc % 2), qk_unit(0, 0, hc))
            for t in range(2, NTILE):                    # b0 V: by t+4
                put(t, v_unit(0, t))
            put(44, qk_unit(1, 0, 0), qk_unit(1, 1, 0))
            put(46, qk_unit(1, 0, 1))
            for hc in range(1, 8):                       # b1 K: by 64+2hc
                put(58 + 2 * (hc - 1) if hc < 4 else 62 + 2 * hc,
                    qk_unit(1, 1, hc))
            for hc in range(2, 8):                       # b1 Q
                put(64 + 16 * (hc // 2) - 10 + 2 * (hc % 2), qk_unit(1, 0, hc))
            for t in range(NTILE):                       # b1 V: by 64+t+4
                put(60 + t if t < 4 else 64 + t, v_unit(1, t))

            for b in range(B):
                for ch in range(4):
                    base = (b * 4 + ch) * 16
                    # pv(ch) fully stopped after next-chunk slot 2
                    put(base + 19, pv_read(b, ch))
                    put(base + 20, normalize(b, ch, 0), normalize(b, ch, 1))
                    put(base + 21, outT_alloc(b, ch),
                        normalize(b, ch, 2), normalize(b, ch, 3))
                    for st in range(4):
                        put(base + 22 + st, transpose_unit(b, ch, st))
                        put(base + 24 + 2 * st, stage_alloc(b, ch, st),
                            outproj_unit(b, ch, st, 0))
                        put(base + 25 + 2 * st, outproj_unit(b, ch, st, 1))
                        put(base + 26 + 2 * st, dma_out(b, ch, st))

            # ---- main exp-paced loop ----
            e2_tiles = {}

            def pv_tile_mm(b, ch, tt):
                """8 accumulation matmuls (h x st) for one exp tile."""
                e2 = e2_tiles.pop((b, ch, tt))
                for h in range(2):
                    for st in range(4):
                        first = (tt == 0 and st == 0)
                        nc.tensor.matmul(
                            pv_ps[h][:, 65 * st:65 * st + 65],
                            e2[:, h, st * 128:(st + 1) * 128],
                            v_sb[:, b, h, tt, :],
                            start=first, stop=(tt == 15 and st == 3),
                            skip_group_check=not first)

            def exp_tile(g, b, ch, tt, sc):
                e2 = e2_pool.tile([128, 2, 512], f16, tag="e2", name=f"e2{g}")
                e2_tiles[(b, ch, tt)] = e2
                if tt in DVE_EXP_TT:
                    nc.vector.tensor_scalar(
                        e2[:, :, :].bitcast(i16), sc[:, :, :],
                        SCHRAU_A, SCHRAU_B, MUL, ADD)
                else:
                    nc.scalar.activation(e2[:, :, :], sc[:, :, :], AF.Exp)

            for u in pre_units:
                u()

            for g in range(NSLOT):
                b, ch, tt = g // 64, (g // 16) % 4, g % 16
                s0 = b * 2048 + ch * 512
                t0 = b * 2048 + tt * 128
                sc = scps_pool.tile([128, 2, 512], f32, tag="sc", name=f"sc{g}")
                for h in range(2):
                    lo = h * 64
                    nc.tensor.matmul(
                        sc[:, h, :],
                        qk_sb[lo:lo + 64, 1, t0:t0 + 128],
                        qk_sb[lo:lo + 64, 0, s0:s0 + 512],
                        start=True, stop=True)
                exp_tile(g, b, ch, tt, sc)
                for u in sched[g]:
                    u()
                # pv lags exp by 3-4 tiles; tt0 is held until slot 4 so the
                # previous chunk's pv_read copies clear the banks first
                if tt == 4:
                    pv_tile_mm(b, ch, 0)
                    pv_tile_mm(b, ch, 1)
                elif tt > 4:
                    pv_tile_mm(b, ch, tt - 3)
                elif tt < 3 and (b, ch) != (0, 0):
                    pb, pch = (b, ch - 1) if ch else (b - 1, 3)
                    pv_tile_mm(pb, pch, 13 + tt)

            for tt in (13, 14, 15):
                pv_tile_mm(1, 3, tt)
            for u in sched[NSLOT]:
                u()
    nc.compile()
    return nc


def _get_nc():
    if "nc" not in _COMPILED:
        _COMPILED["nc"] = _build()
    return _COMPILED["nc"]


def _hi_lo(x, f8):
    hi = x.astype(f8)
    lo = (x - hi.astype(np.float32)).astype(f8)
    return hi, lo


def _tile_dk(x):
    # [1024, N] -> [128, 8, N] with partition p = d % 128, dk = d // 128
    return np.ascontiguousarray(x.reshape(8, 128, -1).transpose(1, 0, 2))


def _prep_inputs(q, in_w, qkv_bias):
    import ml_dtypes
    f8 = ml_dtypes.float8_e4m3
    qT = q.transpose(2, 0, 1).reshape(D, BS)
    qh, ql = _hi_lo(_tile_dk(qT), f8)
    maps = []
    for c in range(NCORES):
        r = slice(128 * c, 128 * (c + 1))
        wq, wk = in_w[0:D][r], in_w[D:2 * D][r]
        wv = in_w[2 * D:3 * D][r]
        wqk = np.concatenate([wq.T, wk.T], axis=1) * 16.0     # [1024, 256]
        wqkh, wqkl = _hi_lo(_tile_dk(wqk), f8)
        wvh, wvl = _hi_lo(_tile_dk(wv.T * 16.0), f8)
        qkb = np.stack([qkv_bias[0:D][r] / 8.0, qkv_bias[D:2 * D][r]], axis=1)
        maps.append({
            "qh": qh, "ql": ql,
            "wqkh": wqkh, "wqkl": wqkl, "wvh": wvh, "wvl": wvl,
            "w2": None,
            "qkb": np.ascontiguousarray(qkb).astype(np.float32),
        })
    return maps


def kernel(q, k, v, in_w, qkv_bias, out_w, out_b, _trace=False):
    from concourse.bass_utils import run_bass_kernel_spmd

    q = np.asarray(q, dtype=np.float32)
    in_w = np.asarray(in_w, dtype=np.float32)
    qkv_bias = np.asarray(qkv_bias, dtype=np.float32)
    out_w = np.asarray(out_w, dtype=np.float32)
    out_b = np.asarray(out_b, dtype=np.float32)

    nc = _get_nc()
    in_maps = _prep_inputs(q, in_w, qkv_bias)
    for c in range(NCORES):
        r = slice(128 * c, 128 * (c + 1))
        in_maps[c]["w2"] = np.ascontiguousarray(out_w[:, r].T).astype(np.float16)

    res = run_bass_kernel_spmd(
        nc, in_maps, core_ids=list(range(NCORES)), trace=_trace,
    )
    total = np.zeros((BS, D), dtype=np.float32)
    for c in range(NCORES):
        total += res.results[c]["partialT"].astype(np.float32)
    net = total + out_b[None, :] + qkv_bias[2 * D:3 * D] @ out_w.T
    out = net.reshape(B, S, D).astype(np.float32)
    if _trace:
        return out, nc, res
    return out


# revision 28
# speedup vs baseline: 1.2844x; 1.0187x over previous
"""Trainium2 Bass kernel for fused self-attention (nn_Attention), v3.

Reference (k/v inputs dead; only q used):
    qkv = q @ in_w.T + qkv_bias ; qp,kp,vp = split(qkv)
    per head: softmax(qp @ kp.T / 8) @ vp ;  net = concat @ out_w.T + out_b

Sharding: tensor-parallel over heads, 2 heads/core. Core c computes its
2-head slice end-to-end plus a partial output projection against its 128
columns of out_w; host sums the 8 partials (plus the V-bias term, which
commutes through softmax, added host-side as vb @ out_w.T).

Cost-model facts driving the design: matmul = out_free x pe_cycle x cpr
(fp16 cpr=1; fp8e4+DoubleRow cpr=0.5 and contracts two k-tiles per call);
ACT exp = 0.833ns/elem/lane -> the ~133us pacing floor; gpsimd cannot
read PSUM. Plain fp8 operands fail the 2e-2 gate (~5e-2 measured), so
fp8 appears only as hi/lo splits (x ~ hi8 + lo8, ~0.3% error):

  - QKV proj: 3-pass hi/lo fp8 DR (qh*wh + ql*wh + qh*wl; lo*lo dropped).
    Host preps q and 16*w hi/lo pairs; descale+bias fold into the
    PSUM->SBUF copies (Q also folds the 1/8 score scale, K/V fold 1/16).
  - scores, per (h, t-tile): DR pass kh*qh + kl*qh (slots [kh,kl] x
    [qh,qh-dup]) + plain-f8 pass kh*ql; ql*kl dropped. 320ns/tile vs
    fp16's 426.
  - exp: ACT, psum [128,2(h),512] -> e2 f16 (a tunable subset of tiles
    runs on DVE via a Schraudolph bit-trick into int16-viewed f16).
  - PV fp16, s-on-partitions: out[s-tile,65] per (h,st), accumulated over
    t-tiles; v_sb [t,65] per head with ones col 64 -> accum col 64 = the
    softmax denominator, already per-partition. The 8 accumulators are
    sub-bank packed into two persistent psum tiles ([128,388] f32: 4
    accums + two [128,128]-f16 transpose slots each). Only the first
    matmul per bank/chunk starts and the last stops: zero-region marking
    is bank-granular, later first-writes land on pending-zero bytes =
    fresh write (validated on HW).
  - normalize: ONE DVE copy per h (psum [128,260] -> pvc f16) is the only
    pv reader that gates the next chunk's accumulation; then DVE
    reciprocal [128,1] per (h,st) + gpsimd multiply (SBUF-only) ->
    norm_sb [s, 128(h0|h1)] f16; PE-transpose into the pv tile tails
    (start=False: those bytes are pending-zero from the current chunk's
    group start); DVE copy -> outT_sb [d, 512] f16.
  - out-proj fp16: lhsT=outT[:,st*128:], rhs=w2 [128,512] -> one psum
    bank; stage copy f16; DMA per s-tile.
  - PSUM: scores 2x2 banks + pv 2x1 + weave 2x1 = 8.
  - Schedule: the 128 exp tiles pace everything; projection/normalize/
    out-proj units are woven into per-tile slots with static deadlines
    (K/Q/V stream ahead of first use, pv lags exp by 3-4 tiles, chunk
    ch's tail rides chunk ch+1).
"""

import sys

for p in ("/opt/trn_rl_repo", "/root/.axon_site/_ro/trn_rl_repo"):
    if p not in sys.path:
        sys.path.append(p)

import numpy as np

B, S, D, H = 2, 2048, 1024, 16
BS = B * S
NCORES = 8
NTILE = 16

_COMPILED = {}

# in-chunk slots whose exp runs on DVE (Schraudolph) instead of ACT
DVE_EXP_TT = ()
SCHRAU_A = 1024.0 / np.log(2.0)
SCHRAU_B = 15.0 * 1024.0 + 0.5 - 29.0


def _build():
    import concourse.bass as bass  # noqa: F401
    import concourse.mybir as mybir
    import concourse.tile as tile
    from concourse import bacc
    from concourse.masks import make_identity

    f8 = mybir.dt.float8e4
    f16 = mybir.dt.float16
    f32 = mybir.dt.float32
    i16 = mybir.dt.int16
    AF = mybir.ActivationFunctionType
    DR = mybir.MatmulPerfMode.DoubleRow
    MUL = mybir.AluOpType.mult
    ADD = mybir.AluOpType.add
    SUB = mybir.AluOpType.subtract

    nc = bacc.Bacc("TRN2", target_bir_lowering=False, debug=False,
                   num_devices=NCORES)

    qh_d = nc.declare_dram_parameter("qh", [128, 8, BS], f8, isOutput=False)
    ql_d = nc.declare_dram_parameter("ql", [128, 8, BS], f8, isOutput=False)
    wqkh_d = nc.declare_dram_parameter("wqkh", [128, 8, 256], f8, isOutput=False)
    wqkl_d = nc.declare_dram_parameter("wqkl", [128, 8, 256], f8, isOutput=False)
    wvh_d = nc.declare_dram_parameter("wvh", [128, 8, 128], f8, isOutput=False)
    wvl_d = nc.declare_dram_parameter("wvl", [128, 8, 128], f8, isOutput=False)
    w2_d = nc.declare_dram_parameter("w2", [128, 1024], f16, isOutput=False)
    qkb_d = nc.declare_dram_parameter("qkb", [128, 2], f32, isOutput=False)
    out_d = nc.declare_dram_parameter("partialT", [BS, 1024], f16, isOutput=True)

    with tile.TileContext(nc) as tc:
        with (
            tc.tile_pool(name="persist", bufs=1) as persist,
            tc.tile_pool(name="e2", bufs=7) as e2_pool,
            tc.tile_pool(name="pvc", bufs=2) as pvc_pool,
            tc.tile_pool(name="r", bufs=8) as r_pool,
            tc.tile_pool(name="norm", bufs=4) as norm_pool,
            tc.tile_pool(name="outT", bufs=2) as outT_pool,
            tc.tile_pool(name="stage", bufs=3) as stage_pool,
            tc.tile_pool(name="scps", bufs=2, space="PSUM") as scps_pool,
            tc.tile_pool(name="pvps", bufs=2, space="PSUM") as pvps_pool,
            tc.tile_pool(name="wvps", bufs=2, space="PSUM") as wvps_pool,
        ):
            qh_sb = persist.tile([128, 8, BS], f8)      # 32KB/part
            ql_sb = persist.tile([128, 8, BS], f8)      # 32KB/part
            wqkh_sb = persist.tile([128, 8, 256], f8)
            wqkl_sb = persist.tile([128, 8, 256], f8)
            wvh_sb = persist.tile([128, 8, 128], f8)
            wvl_sb = persist.tile([128, 8, 128], f8)
            w2_sb = persist.tile([128, 1024], f16)
            qkb_sb = persist.tile([128, 2], f32)
            # projected Q/8 (m=0) and K (m=1), partitions h*64+d
            qk_sb = persist.tile([128, 2, BS], f16)     # 16KB/part
            v_sb = persist.tile([128, B, 2, NTILE, 65], f16)  # 16.3KB/part
            ident_sb = persist.tile([128, 128], f16)
            dummy_sb = persist.tile([128, 128], f8)
            warm_sb = persist.tile([1, 8], f32)

            # persistent pv psum tiles (one bank each): 4 accums [128,65]
            pv_ps = [pvps_pool.tile([128, 260], f32, tag="pv", name=f"pv{h}")
                     for h in range(2)]

            # ---- preamble ----
            nc.vector.memset(warm_sb[:, :], 0.0)
            nc.scalar.activation(warm_sb[:, :], warm_sb[:, :], AF.Exp)
            nc.vector.memset(dummy_sb[:, :], 0.0)
            make_identity(nc, ident_sb[:, :])
            for b in range(B):
                nc.gpsimd.memset(v_sb[:, b, :, :, 64:65], 1.0)

            # loads ordered by first use: qk weights + bias, then q chunk 0,
            # then V weights / w2 / remaining q chunks
            nc.sync.dma_start(wqkh_sb[:, :, :], wqkh_d[:, :, :])
            nc.sync.dma_start(wqkl_sb[:, :, :], wqkl_d[:, :, :])
            nc.sync.dma_start(qkb_sb[:, :], qkb_d[:, :])
            nc.sync.dma_start(qh_sb[:, :, 0:512], qh_d[:, :, 0:512])
            nc.sync.dma_start(ql_sb[:, :, 0:512], ql_d[:, :, 0:512])
            nc.sync.dma_start(wvh_sb[:, :, :], wvh_d[:, :, :])
            nc.sync.dma_start(wvl_sb[:, :, :], wvl_d[:, :, :])
            nc.sync.dma_start(w2_sb[:, :], w2_d[:, :])
            for scc in range(1, 8):
                nc.sync.dma_start(qh_sb[:, :, scc * 512:(scc + 1) * 512],
                                  qh_d[:, :, scc * 512:(scc + 1) * 512])
                nc.sync.dma_start(ql_sb[:, :, scc * 512:(scc + 1) * 512],
                                  ql_d[:, :, scc * 512:(scc + 1) * 512])

            wps = wvps_pool.tile([128, 512], f32, tag="wv", name="warm")
            for i in range(48):
                nc.tensor.matmul(wps[:, 0:128], dummy_sb[:, :], dummy_sb[:, :],
                                 start=True, stop=True)

            # ---- work units ----
            def qk_unit(b, m, hc):
                """Q (m=0) / K (m=1) projection, one 256-col half-chunk:
                3 hi/lo DR passes + descale/bias copies into q8/k8 slots."""
                def emit():
                    s0 = b * 2048 + hc * 256
                    ps = wvps_pool.tile([128, 512], f32, tag="wv",
                                        name=f"qk{b}{m}{hc}")
                    ops = [(qh_sb, wqkh_sb), (ql_sb, wqkh_sb), (qh_sb, wqkl_sb)]
                    for pi, (qs, ws) in enumerate(ops):
                        for j in range(4):
                            nc.tensor.matmul(
                                ps[:, 0:256],
                                ws[:, 2 * j:2 * j + 2, m * 128:(m + 1) * 128],
                                qs[:, 2 * j:2 * j + 2, s0:s0 + 256],
                                start=(pi == 0 and j == 0),
                                stop=(pi == 2 and j == 3), perf_mode=DR)
                    dsc = (1.0 / 128.0) if m == 0 else (1.0 / 16.0)
                    nc.vector.tensor_scalar(
                        qk_sb[:, m, s0:s0 + 256], ps[:, 0:256],
                        dsc, qkb_sb[:, m:m + 1], MUL, ADD)
                return emit

            def v_unit(b, t):
                """V projection for one t-tile, direct [t, dv]: 3 hi/lo DR
                passes + one strided descale copy into both head slots."""
                def emit():
                    t0 = b * 2048 + t * 128
                    ps = wvps_pool.tile([128, 512], f32, tag="wv",
                                        name=f"v{b}{t}")
                    ops = [(qh_sb, wvh_sb), (ql_sb, wvh_sb), (qh_sb, wvl_sb)]
                    for pi, (qs, ws) in enumerate(ops):
                        for j in range(4):
                            nc.tensor.matmul(
                                ps[:, 0:128],
                                qs[:, 2 * j:2 * j + 2, t0:t0 + 128],
                                ws[:, 2 * j:2 * j + 2, :],
                                start=(pi == 0 and j == 0),
                                stop=(pi == 2 and j == 3), perf_mode=DR)
                    nc.vector.tensor_scalar(
                        v_sb[:, b, :, t, 0:64], ps[:, 0:128],
                        1.0 / 16.0, None, MUL)
                return emit

            pvc_tiles = {}
            norm_tiles = {}
            outT_tiles = {}
            stage_tiles = {}

            def pv_read(b, ch):
                """The only pv psum readers: one [128,260] f32->f16 copy per
                head. Everything downstream works from SBUF."""
                def emit():
                    for h in range(2):
                        t = pvc_pool.tile([128, 4, 65], f16, tag="pvc",
                                          name=f"pvc{b}{ch}{h}")
                        pvc_tiles[(b, ch, h)] = t
                        nc.vector.tensor_copy(t[:, :, :], pv_ps[h][:, 0:260])
                return emit

            def normalize(b, ch, st, direct=False):
                """direct=True (last chunk only): read the pv psum straight
                on DVE — no pvc hop, no Pool — to shorten the tail chain."""
                def emit():
                    nm = norm_pool.tile([128, 128], f16, tag="nm",
                                        name=f"nm{b}{ch}{st}")
                    norm_tiles[(b, ch, st)] = nm
                    for h in range(2):
                        r = r_pool.tile([128, 1], f32, tag="r",
                                        name=f"r{b}{ch}{h}{st}")
                        if direct:
                            src = pv_ps[h][:, 65 * st:65 * st + 65]
                        else:
                            src = pvc_tiles[(b, ch, h)][:, st, :]
                        nc.vector.reciprocal(r[:, :], src[:, 64:65])
                        eng = nc.vector if direct else nc.gpsimd
                        eng.tensor_scalar(
                            nm[:, h * 64:(h + 1) * 64],
                            src[:, 0:64], r[:, 0:1], None, MUL)
                return emit

            def transpose_unit(b, ch, st):
                """norm [s,128] -> [d,s] via PE transpose (own psum group),
                then DVE copy into outT_sb. The last chunk borrows the
                idle scps banks so wvps stays free for its out-proj flow."""
                def emit():
                    if (b, ch) == (1, 3):
                        ps = scps_pool.tile([128, 2, 512], f32, tag="sc",
                                            name=f"tr{b}{ch}{st}")[:, 0, :]
                    else:
                        ps = wvps_pool.tile([128, 512], f32, tag="wv",
                                            name=f"tr{b}{ch}{st}")
                    tr = ps[:, 0:64].bitcast(f16)
                    nc.tensor.transpose(tr[:, :], norm_tiles[(b, ch, st)][:, :],
                                        ident_sb[:, :])
                    nc.vector.tensor_copy(
                        outT_tiles[(b, ch)][:, st * 128:(st + 1) * 128],
                        tr[:, :])
                return emit

            def outT_alloc(b, ch):
                def emit():
                    outT_tiles[(b, ch)] = outT_pool.tile(
                        [128, 512], f16, tag="oT", name=f"oT{b}{ch}")
                return emit

            def outproj_unit(b, ch, st, op):
                def emit():
                    ps = wvps_pool.tile([128, 512], f32, tag="wv",
                                        name=f"op{b}{ch}{st}{op}")
                    nc.tensor.matmul(
                        ps[:, :],
                        outT_tiles[(b, ch)][:, st * 128:(st + 1) * 128],
                        w2_sb[:, op * 512:(op + 1) * 512],
                        start=True, stop=True)
                    stg = stage_tiles[(b, ch, st)]
                    if (b, ch) == (1, 3) and op == 1:
                        # ACT is idle after the last exp; split the tail's
                        # stage copies across ACT and DVE
                        nc.scalar.activation(stg[:, 512:1024], ps[:, :],
                                             AF.Copy)
                    else:
                        nc.vector.tensor_copy(stg[:, op * 512:(op + 1) * 512],
                                              ps[:, :])
                return emit

            def stage_alloc(b, ch, st):
                def emit():
                    stage_tiles[(b, ch, st)] = stage_pool.tile(
                        [128, 1024], f16, tag="st", name=f"st{b}{ch}{st}")
                return emit

            def dma_out(b, ch, st):
                def emit():
                    t0 = b * 2048 + ch * 512 + st * 128
                    nc.sync.dma_start(out_d[t0:t0 + 128, :],
                                      stage_tiles[(b, ch, st)][:, :])
                return emit

            # ---- static weave schedule ----
            NSLOT = B * 4 * NTILE
            sched = {i: [] for i in range(NSLOT + 1)}

            def put(g, *fns):
                sched[max(0, min(g, NSLOT))].extend(fns)

            pre_units = [qk_unit(0, 1, 0), qk_unit(0, 0, 0), qk_unit(0, 0, 1),
                         v_unit(0, 0), v_unit(0, 1)]
            for hc in range(1, 8):                       # b0 K: by slot 2hc
                put(2 * hc - 2, qk_unit(0, 1, hc))
            for hc in range(2, 8):                       # b0 Q: by 16*(hc//2)
                put(16 * (hc // 2) - 8 + 2 * (hc % 2), qk_unit(0, 0, hc))
            for t in range(2, NTILE):                    # b0 V: by t+4
                put(t, v_unit(0, t))
            put(44, qk_unit(1, 0, 0), qk_unit(1, 1, 0))
            put(46, qk_unit(1, 0, 1))
            for hc in range(1, 8):                       # b1 K: by 64+2hc
                put(58 + 2 * (hc - 1) if hc < 4 else 62 + 2 * hc,
                    qk_unit(1, 1, hc))
            for hc in range(2, 8):                       # b1 Q
                put(64 + 16 * (hc // 2) - 10 + 2 * (hc % 2), qk_unit(1, 0, hc))
            for t in range(NTILE):                       # b1 V: by 64+t+4
                put(60 + t if t < 4 else 64 + t, v_unit(1, t))

            for b in range(B):
                for ch in range(4):
                    base = (b * 4 + ch) * 16
                    last = (b, ch) == (1, 3)
                    # pv(ch) fully stopped after next-chunk slot 1
                    if not last:
                        put(base + 18, pv_read(b, ch))
                    put(base + 19, normalize(b, ch, 0, last),
                        normalize(b, ch, 1, last))
                    put(base + 20, outT_alloc(b, ch),
                        normalize(b, ch, 2, last), normalize(b, ch, 3, last))
                    for st in range(4):
                        put(base + 20 + st, transpose_unit(b, ch, st))
                        put(base + 22 + 2 * st, stage_alloc(b, ch, st),
                            outproj_unit(b, ch, st, 0))
                        put(base + 23 + 2 * st, outproj_unit(b, ch, st, 1))
                        put(base + 24 + 2 * st, dma_out(b, ch, st))

            # ---- main exp-paced loop ----
            e2_tiles = {}

            def pv_tile_mm(b, ch, tt):
                """8 accumulation matmuls (h x st) for one exp tile."""
                e2 = e2_tiles.pop((b, ch, tt))
                for h in range(2):
                    for st in range(4):
                        first = (tt == 0 and st == 0)
                        nc.tensor.matmul(
                            pv_ps[h][:, 65 * st:65 * st + 65],
                            e2[:, h, st * 128:(st + 1) * 128],
                            v_sb[:, b, h, tt, :],
                            start=first, stop=(tt == 15 and st == 3),
                            skip_group_check=not first)

            def exp_tile(g, b, ch, tt, sc):
                e2 = e2_pool.tile([128, 2, 512], f16, tag="e2", name=f"e2{g}")
                e2_tiles[(b, ch, tt)] = e2
                if tt in DVE_EXP_TT:
                    nc.vector.tensor_scalar(
                        e2[:, :, :].bitcast(i16), sc[:, :, :],
                        SCHRAU_A, SCHRAU_B, MUL, ADD)
                else:
                    nc.scalar.activation(e2[:, :, :], sc[:, :, :], AF.Exp)

            for u in pre_units:
                u()

            for g in range(NSLOT):
                b, ch, tt = g // 64, (g // 16) % 4, g % 16
                s0 = b * 2048 + ch * 512
                t0 = b * 2048 + tt * 128
                sc = scps_pool.tile([128, 2, 512], f32, tag="sc", name=f"sc{g}")
                for h in range(2):
                    lo = h * 64
                    nc.tensor.matmul(
                        sc[:, h, :],
                        qk_sb[lo:lo + 64, 1, t0:t0 + 128],
                        qk_sb[lo:lo + 64, 0, s0:s0 + 512],
                        start=True, stop=True)
                exp_tile(g, b, ch, tt, sc)
                for u in sched[g]:
                    u()
                # pv lags exp by ~3-5 tiles, shrinking toward the chunk end
                # so accumulation stops by next-chunk slot 1; tt0-2 are held
                # until slot 5 so the pv_read copies clear the banks first
                if tt == 5:
                    for t2 in (0, 1, 2):
                        pv_tile_mm(b, ch, t2)
                elif tt > 5:
                    pv_tile_mm(b, ch, tt - 3)
                elif (b, ch) != (0, 0):
                    pb, pch = (b, ch - 1) if ch else (b - 1, 3)
                    if tt == 0:
                        pv_tile_mm(pb, pch, 13)
                        pv_tile_mm(pb, pch, 14)
                    elif tt == 1:
                        pv_tile_mm(pb, pch, 15)

            for t2 in (13, 14, 15):
                pv_tile_mm(1, 3, t2)
            for u in sched[NSLOT]:
                u()
    nc.compile()
    return nc


def _get_nc():
    if "nc" not in _COMPILED:
        _COMPILED["nc"] = _build()
    return _COMPILED["nc"]


def _hi_lo(x, f8):
    hi = x.astype(f8)
    lo = (x - hi.astype(np.float32)).astype(f8)
    return hi, lo


def _tile_dk(x):
    # [1024, N] -> [128, 8, N] with partition p = d % 128, dk = d // 128
    return np.ascontiguousarray(x.reshape(8, 128, -1).transpose(1, 0, 2))


def _prep_inputs(q, in_w, qkv_bias):
    import ml_dtypes
    f8 = ml_dtypes.float8_e4m3
    qT = q.transpose(2, 0, 1).reshape(D, BS)
    qh, ql = _hi_lo(_tile_dk(qT), f8)
    maps = []
    for c in range(NCORES):
        r = slice(128 * c, 128 * (c + 1))
        wq, wk = in_w[0:D][r], in_w[D:2 * D][r]
        wv = in_w[2 * D:3 * D][r]
        wqk = np.concatenate([wq.T, wk.T], axis=1) * 16.0     # [1024, 256]
        wqkh, wqkl = _hi_lo(_tile_dk(wqk), f8)
        wvh, wvl = _hi_lo(_tile_dk(wv.T * 16.0), f8)
        qkb = np.stack([qkv_bias[0:D][r] / 8.0, qkv_bias[D:2 * D][r]], axis=1)
        maps.append({
            "qh": qh, "ql": ql,
            "wqkh": wqkh, "wqkl": wqkl, "wvh": wvh, "wvl": wvl,
            "w2": None,
            "qkb": np.ascontiguousarray(qkb).astype(np.float32),
        })
    return maps


def kernel(q, k, v, in_w, qkv_bias, out_w, out_b, _trace=False):
    from concourse.bass_utils import run_bass_kernel_spmd

    q = np.asarray(q, dtype=np.float32)
    in_w = np.asarray(in_w, dtype=np.float32)
    qkv_bias = np.asarray(qkv_bias, dtype=np.float32)
    out_w = np.asarray(out_w, dtype=np.float32)
    out_b = np.asarray(out_b, dtype=np.float32)

    nc = _get_nc()
    in_maps = _prep_inputs(q, in_w, qkv_bias)
    for c in range(NCORES):
        r = slice(128 * c, 128 * (c + 1))
        in_maps[c]["w2"] = np.ascontiguousarray(out_w[:, r].T).astype(np.float16)

    res = run_bass_kernel_spmd(
        nc, in_maps, core_ids=list(range(NCORES)), trace=_trace,
    )
    total = np.zeros((BS, D), dtype=np.float32)
    for c in range(NCORES):
        total += res.results[c]["partialT"].astype(np.float32)
    net = total + out_b[None, :] + qkv_bias[2 * D:3 * D] @ out_w.T
    out = net.reshape(B, S, D).astype(np.float32)
    if _trace:
        return out, nc, res
    return out


# revision 36
# speedup vs baseline: 1.2874x; 1.0024x over previous
"""Trainium2 Bass kernel for fused self-attention (nn_Attention), v3.

Reference (k/v inputs dead; only q used):
    qkv = q @ in_w.T + qkv_bias ; qp,kp,vp = split(qkv)
    per head: softmax(qp @ kp.T / 8) @ vp ;  net = concat @ out_w.T + out_b

Sharding: tensor-parallel over heads, 2 heads/core. Core c computes its
2-head slice end-to-end plus a partial output projection against its 128
columns of out_w; host sums the 8 partials (plus the V-bias term, which
commutes through softmax, added host-side as vb @ out_w.T).

Cost-model facts driving the design: matmul = out_free x pe_cycle x cpr
(fp16 cpr=1; fp8e4+DoubleRow cpr=0.5 and contracts two k-tiles per call);
ACT exp = 0.833ns/elem/lane -> the ~133us pacing floor; gpsimd cannot
read PSUM. Plain fp8 operands fail the 2e-2 gate (~5e-2 measured), so
fp8 appears only as hi/lo splits (x ~ hi8 + lo8, ~0.3% error):

  - QKV proj: 3-pass hi/lo fp8 DR (qh*wh + ql*wh + qh*wl; lo*lo dropped).
    Host preps q and 16*w hi/lo pairs; descale+bias fold into the
    PSUM->SBUF copies (Q also folds the 1/8 score scale, K/V fold 1/16).
  - scores, per (h, t-tile): DR pass kh*qh + kl*qh (slots [kh,kl] x
    [qh,qh-dup]) + plain-f8 pass kh*ql; ql*kl dropped. 320ns/tile vs
    fp16's 426.
  - exp: ACT, psum [128,2(h),512] -> e2 f16 (a tunable subset of tiles
    runs on DVE via a Schraudolph bit-trick into int16-viewed f16).
  - PV fp16, s-on-partitions: out[s-tile,65] per (h,st), accumulated over
    t-tiles; v_sb [t,65] per head with ones col 64 -> accum col 64 = the
    softmax denominator, already per-partition. The 8 accumulators are
    sub-bank packed into two persistent psum tiles ([128,388] f32: 4
    accums + two [128,128]-f16 transpose slots each). Only the first
    matmul per bank/chunk starts and the last stops: zero-region marking
    is bank-granular, later first-writes land on pending-zero bytes =
    fresh write (validated on HW).
  - normalize: ONE DVE copy per h (psum [128,260] -> pvc f16) is the only
    pv reader that gates the next chunk's accumulation; then DVE
    reciprocal [128,1] per (h,st) + gpsimd multiply (SBUF-only) ->
    norm_sb [s, 128(h0|h1)] f16; PE-transpose into the pv tile tails
    (start=False: those bytes are pending-zero from the current chunk's
    group start); DVE copy -> outT_sb [d, 512] f16.
  - out-proj fp16: lhsT=outT[:,st*128:], rhs=w2 [128,512] -> one psum
    bank; stage copy f16; DMA per s-tile.
  - PSUM: scores 2x2 banks + pv 2x1 + weave 2x1 = 8.
  - Schedule: the 128 exp tiles pace everything; projection/normalize/
    out-proj units are woven into per-tile slots with static deadlines
    (K/Q/V stream ahead of first use, pv lags exp by 3-4 tiles, chunk
    ch's tail rides chunk ch+1).
"""

import sys

for p in ("/opt/trn_rl_repo", "/root/.axon_site/_ro/trn_rl_repo"):
    if p not in sys.path:
        sys.path.append(p)

import numpy as np

B, S, D, H = 2, 2048, 1024, 16
BS = B * S
NCORES = 8
NTILE = 16

_COMPILED = {}

# in-chunk slots whose exp runs on DVE (Schraudolph) instead of ACT
DVE_EXP_TT = ()
SCHRAU_A = 1024.0 / np.log(2.0)
SCHRAU_B = 15.0 * 1024.0 + 0.5 - 29.0


def _build():
    import concourse.bass as bass  # noqa: F401
    import concourse.mybir as mybir
    import concourse.tile as tile
    from concourse import bacc
    from concourse.masks import make_identity

    f8 = mybir.dt.float8e4
    f16 = mybir.dt.float16
    f32 = mybir.dt.float32
    i16 = mybir.dt.int16
    AF = mybir.ActivationFunctionType
    DR = mybir.MatmulPerfMode.DoubleRow
    MUL = mybir.AluOpType.mult
    ADD = mybir.AluOpType.add
    SUB = mybir.AluOpType.subtract

    nc = bacc.Bacc("TRN2", target_bir_lowering=False, debug=False,
                   num_devices=NCORES)

    qh_d = nc.declare_dram_parameter("qh", [128, 8, BS], f8, isOutput=False)
    ql_d = nc.declare_dram_parameter("ql", [128, 8, BS], f8, isOutput=False)
    wqkh_d = nc.declare_dram_parameter("wqkh", [128, 8, 256], f8, isOutput=False)
    wqkl_d = nc.declare_dram_parameter("wqkl", [128, 8, 256], f8, isOutput=False)
    wvh_d = nc.declare_dram_parameter("wvh", [128, 8, 128], f8, isOutput=False)
    wvl_d = nc.declare_dram_parameter("wvl", [128, 8, 128], f8, isOutput=False)
    w2_d = nc.declare_dram_parameter("w2", [128, 1024], f16, isOutput=False)
    qkb_d = nc.declare_dram_parameter("qkb", [128, 2], f32, isOutput=False)
    out_d = nc.declare_dram_parameter("partialT", [BS, 1024], f16, isOutput=True)

    with tile.TileContext(nc) as tc:
        with (
            tc.tile_pool(name="persist", bufs=1) as persist,
            tc.tile_pool(name="e2", bufs=7) as e2_pool,
            tc.tile_pool(name="pvc", bufs=2) as pvc_pool,
            tc.tile_pool(name="r", bufs=8) as r_pool,
            tc.tile_pool(name="norm", bufs=4) as norm_pool,
            tc.tile_pool(name="outT", bufs=2) as outT_pool,
            tc.tile_pool(name="stage", bufs=3) as stage_pool,
            tc.tile_pool(name="scps", bufs=2, space="PSUM") as scps_pool,
            tc.tile_pool(name="pvps", bufs=2, space="PSUM") as pvps_pool,
            tc.tile_pool(name="wvps", bufs=2, space="PSUM") as wvps_pool,
        ):
            qh_sb = persist.tile([128, 8, BS], f8)      # 32KB/part
            ql_sb = persist.tile([128, 8, BS], f8)      # 32KB/part
            wqkh_sb = persist.tile([128, 8, 256], f8)
            wqkl_sb = persist.tile([128, 8, 256], f8)
            wvh_sb = persist.tile([128, 8, 128], f8)
            wvl_sb = persist.tile([128, 8, 128], f8)
            w2_sb = persist.tile([128, 1024], f16)
            qkb_sb = persist.tile([128, 2], f32)
            # projected Q/8 (m=0) and K (m=1), partitions h*64+d
            qk_sb = persist.tile([128, 2, BS], f16)     # 16KB/part
            v_sb = persist.tile([128, B, 2, NTILE, 65], f16)  # 16.3KB/part
            ident_sb = persist.tile([128, 128], f16)
            dummy_sb = persist.tile([128, 128], f8)
            warm_sb = persist.tile([1, 8], f32)

            # persistent pv psum tiles (one bank each): 4 accums [128,65]
            pv_ps = [pvps_pool.tile([128, 260], f32, tag="pv", name=f"pv{h}")
                     for h in range(2)]

            # ---- preamble ----
            nc.vector.memset(warm_sb[:, :], 0.0)
            nc.scalar.activation(warm_sb[:, :], warm_sb[:, :], AF.Exp)
            nc.vector.memset(dummy_sb[:, :], 0.0)
            make_identity(nc, ident_sb[:, :])
            for b in range(B):
                nc.gpsimd.memset(v_sb[:, b, :, :, 64:65], 1.0)

            # loads ordered by first use: the first K/Q units only touch q
            # cols 0:256, so those quarter-pieces go first with the qk
            # weights; V weights / w2 / remaining q chunks follow
            nc.sync.dma_start(wqkh_sb[:, :, :], wqkh_d[:, :, :])
            nc.sync.dma_start(wqkl_sb[:, :, :], wqkl_d[:, :, :])
            nc.sync.dma_start(qkb_sb[:, :], qkb_d[:, :])
            nc.sync.dma_start(qh_sb[:, :, 0:512], qh_d[:, :, 0:512])
            nc.sync.dma_start(ql_sb[:, :, 0:512], ql_d[:, :, 0:512])
            nc.sync.dma_start(wvh_sb[:, :, :], wvh_d[:, :, :])
            nc.sync.dma_start(wvl_sb[:, :, :], wvl_d[:, :, :])
            nc.sync.dma_start(w2_sb[:, :], w2_d[:, :])
            for scc in range(1, 8):
                nc.sync.dma_start(qh_sb[:, :, scc * 512:(scc + 1) * 512],
                                  qh_d[:, :, scc * 512:(scc + 1) * 512])
                nc.sync.dma_start(ql_sb[:, :, scc * 512:(scc + 1) * 512],
                                  ql_d[:, :, scc * 512:(scc + 1) * 512])

            wps = wvps_pool.tile([128, 512], f32, tag="wv", name="warm")
            for i in range(48):
                nc.tensor.matmul(wps[:, 0:128], dummy_sb[:, :], dummy_sb[:, :],
                                 start=True, stop=True)

            # ---- work units ----
            def qk_unit(b, m, hc):
                """Q (m=0) / K (m=1) projection, one 256-col half-chunk:
                3 hi/lo DR passes + descale/bias copies into q8/k8 slots."""
                def emit():
                    s0 = b * 2048 + hc * 256
                    ps = wvps_pool.tile([128, 512], f32, tag="wv",
                                        name=f"qk{b}{m}{hc}")
                    ops = [(qh_sb, wqkh_sb), (ql_sb, wqkh_sb), (qh_sb, wqkl_sb)]
                    for pi, (qs, ws) in enumerate(ops):
                        for j in range(4):
                            nc.tensor.matmul(
                                ps[:, 0:256],
                                ws[:, 2 * j:2 * j + 2, m * 128:(m + 1) * 128],
                                qs[:, 2 * j:2 * j + 2, s0:s0 + 256],
                                start=(pi == 0 and j == 0),
                                stop=(pi == 2 and j == 3), perf_mode=DR)
                    dsc = (1.0 / 128.0) if m == 0 else (1.0 / 16.0)
                    nc.vector.tensor_scalar(
                        qk_sb[:, m, s0:s0 + 256], ps[:, 0:256],
                        dsc, qkb_sb[:, m:m + 1], MUL, ADD)
                return emit

            def v_unit(b, t):
                """V projection for one t-tile, direct [t, dv]: 3 hi/lo DR
                passes + one strided descale copy into both head slots."""
                def emit():
                    t0 = b * 2048 + t * 128
                    ps = wvps_pool.tile([128, 512], f32, tag="wv",
                                        name=f"v{b}{t}")
                    ops = [(qh_sb, wvh_sb), (ql_sb, wvh_sb), (qh_sb, wvl_sb)]
                    for pi, (qs, ws) in enumerate(ops):
                        for j in range(4):
                            nc.tensor.matmul(
                                ps[:, 0:128],
                                qs[:, 2 * j:2 * j + 2, t0:t0 + 128],
                                ws[:, 2 * j:2 * j + 2, :],
                                start=(pi == 0 and j == 0),
                                stop=(pi == 2 and j == 3), perf_mode=DR)
                    nc.vector.tensor_scalar(
                        v_sb[:, b, :, t, 0:64], ps[:, 0:128],
                        1.0 / 16.0, None, MUL)
                return emit

            pvc_tiles = {}
            norm_tiles = {}
            outT_tiles = {}
            stage_tiles = {}

            def pv_read(b, ch):
                """The only pv psum readers: one [128,260] f32->f16 copy per
                head. Everything downstream works from SBUF."""
                def emit():
                    for h in range(2):
                        t = pvc_pool.tile([128, 4, 65], f16, tag="pvc",
                                          name=f"pvc{b}{ch}{h}")
                        pvc_tiles[(b, ch, h)] = t
                        nc.vector.tensor_copy(t[:, :, :], pv_ps[h][:, 0:260])
                return emit

            def normalize(b, ch, st, direct=False):
                """direct=True (last chunk only): read the pv psum straight
                on DVE — no pvc hop, no Pool — to shorten the tail chain."""
                def emit():
                    nm = norm_pool.tile([128, 128], f16, tag="nm",
                                        name=f"nm{b}{ch}{st}")
                    norm_tiles[(b, ch, st)] = nm
                    for h in range(2):
                        r = r_pool.tile([128, 1], f32, tag="r",
                                        name=f"r{b}{ch}{h}{st}")
                        if direct:
                            src = pv_ps[h][:, 65 * st:65 * st + 65]
                        else:
                            src = pvc_tiles[(b, ch, h)][:, st, :]
                        nc.vector.reciprocal(r[:, :], src[:, 64:65])
                        eng = nc.vector if direct else nc.gpsimd
                        eng.tensor_scalar(
                            nm[:, h * 64:(h + 1) * 64],
                            src[:, 0:64], r[:, 0:1], None, MUL)
                return emit

            def transpose_unit(b, ch, st):
                """norm [s,128] -> [d,s] via PE transpose (own psum group),
                then DVE copy into outT_sb. The last chunk borrows the
                idle scps banks so wvps stays free for its out-proj flow."""
                def emit():
                    if (b, ch) == (1, 3):
                        ps = scps_pool.tile([128, 2, 512], f32, tag="sc",
                                            name=f"tr{b}{ch}{st}")[:, 0, :]
                    else:
                        ps = wvps_pool.tile([128, 512], f32, tag="wv",
                                            name=f"tr{b}{ch}{st}")
                    tr = ps[:, 0:64].bitcast(f16)
                    nc.tensor.transpose(tr[:, :], norm_tiles[(b, ch, st)][:, :],
                                        ident_sb[:, :])
                    nc.vector.tensor_copy(
                        outT_tiles[(b, ch)][:, st * 128:(st + 1) * 128],
                        tr[:, :])
                return emit

            def outT_alloc(b, ch):
                def emit():
                    outT_tiles[(b, ch)] = outT_pool.tile(
                        [128, 512], f16, tag="oT", name=f"oT{b}{ch}")
                return emit

            def outproj_unit(b, ch, st, op):
                def emit():
                    ps = wvps_pool.tile([128, 512], f32, tag="wv",
                                        name=f"op{b}{ch}{st}{op}")
                    nc.tensor.matmul(
                        ps[:, :],
                        outT_tiles[(b, ch)][:, st * 128:(st + 1) * 128],
                        w2_sb[:, op * 512:(op + 1) * 512],
                        start=True, stop=True)
                    stg = stage_tiles[(b, ch, st)]
                    if (b, ch) == (1, 3) and op == 1:
                        # ACT is idle after the last exp; split the tail's
                        # stage copies across ACT and DVE
                        nc.scalar.activation(stg[:, 512:1024], ps[:, :],
                                             AF.Copy)
                    else:
                        nc.vector.tensor_copy(stg[:, op * 512:(op + 1) * 512],
                                              ps[:, :])
                return emit

            def stage_alloc(b, ch, st):
                def emit():
                    stage_tiles[(b, ch, st)] = stage_pool.tile(
                        [128, 1024], f16, tag="st", name=f"st{b}{ch}{st}")
                return emit

            def dma_out(b, ch, st, op=None):
                def emit():
                    t0 = b * 2048 + ch * 512 + st * 128
                    if op is None:
                        nc.sync.dma_start(out_d[t0:t0 + 128, :],
                                          stage_tiles[(b, ch, st)][:, :])
                    else:  # half-stage store (tail: start each ASAP)
                        nc.sync.dma_start(
                            out_d[t0:t0 + 128, op * 512:(op + 1) * 512],
                            stage_tiles[(b, ch, st)][:, op * 512:(op + 1) * 512])
                return emit

            # ---- static weave schedule ----
            NSLOT = B * 4 * NTILE
            sched = {i: [] for i in range(NSLOT + 1)}

            def put(g, *fns):
                sched[max(0, min(g, NSLOT))].extend(fns)

            pre_units = [qk_unit(0, 1, 0), qk_unit(0, 0, 0), qk_unit(0, 0, 1),
                         v_unit(0, 0), v_unit(0, 1)]
            for hc in range(1, 8):                       # b0 K: by slot 2hc
                put(2 * hc - 2, qk_unit(0, 1, hc))
            for hc in range(2, 8):                       # b0 Q: by 16*(hc//2)
                put(16 * (hc // 2) - 8 + 2 * (hc % 2), qk_unit(0, 0, hc))
            for t in range(2, NTILE):                    # b0 V: by t+4
                put(t, v_unit(0, t))
            put(44, qk_unit(1, 0, 0), qk_unit(1, 1, 0))
            put(46, qk_unit(1, 0, 1))
            for hc in range(1, 8):                       # b1 K: by 64+2hc
                put(58 + 2 * (hc - 1) if hc < 4 else 62 + 2 * hc,
                    qk_unit(1, 1, hc))
            for hc in range(2, 8):                       # b1 Q
                put(64 + 16 * (hc // 2) - 10 + 2 * (hc % 2), qk_unit(1, 0, hc))
            for t in range(NTILE):                       # b1 V: by 64+t+4
                put(60 + t if t < 4 else 64 + t, v_unit(1, t))

            for b in range(B):
                for ch in range(4):
                    base = (b * 4 + ch) * 16
                    last = (b, ch) == (1, 3)
                    # pv(ch) fully stopped after next-chunk slot 1
                    if not last:
                        put(base + 18, pv_read(b, ch))
                    put(base + 19, normalize(b, ch, 0, last),
                        normalize(b, ch, 1, last))
                    put(base + 20, outT_alloc(b, ch),
                        normalize(b, ch, 2, last), normalize(b, ch, 3, last))
                    for st in range(4):
                        put(base + 20 + st, transpose_unit(b, ch, st))
                        put(base + 22 + 2 * st, stage_alloc(b, ch, st),
                            outproj_unit(b, ch, st, 0))
                        put(base + 23 + 2 * st, outproj_unit(b, ch, st, 1))
                        put(base + 24 + 2 * st, dma_out(b, ch, st))

            # ---- main exp-paced loop ----
            e2_tiles = {}

            def pv_tile_mm(b, ch, tt):
                """8 accumulation matmuls (h x st) for one exp tile."""
                e2 = e2_tiles.pop((b, ch, tt))
                for h in range(2):
                    for st in range(4):
                        first = (tt == 0 and st == 0)
                        nc.tensor.matmul(
                            pv_ps[h][:, 65 * st:65 * st + 65],
                            e2[:, h, st * 128:(st + 1) * 128],
                            v_sb[:, b, h, tt, :],
                            start=first, stop=(tt == 15 and st == 3),
                            skip_group_check=not first)

            def exp_tile(g, b, ch, tt, sc):
                e2 = e2_pool.tile([128, 2, 512], f16, tag="e2", name=f"e2{g}")
                e2_tiles[(b, ch, tt)] = e2
                if tt in DVE_EXP_TT:
                    nc.vector.tensor_scalar(
                        e2[:, :, :].bitcast(i16), sc[:, :, :],
                        SCHRAU_A, SCHRAU_B, MUL, ADD)
                else:
                    nc.scalar.activation(e2[:, :, :], sc[:, :, :], AF.Exp)

            def emit_scores(g):
                b, ch, tt = g // 64, (g // 16) % 4, g % 16
                s0 = b * 2048 + ch * 512
                t0 = b * 2048 + tt * 128
                sc = scps_pool.tile([128, 2, 512], f32, tag="sc",
                                    name=f"sc{g}")
                for h in range(2):
                    lo = h * 64
                    nc.tensor.matmul(
                        sc[:, h, :],
                        qk_sb[lo:lo + 64, 1, t0:t0 + 128],
                        qk_sb[lo:lo + 64, 0, s0:s0 + 512],
                        start=True, stop=True)
                return sc

            for u in pre_units:
                u()

            # scores run one slot ahead of exp so exp(g) never waits behind
            # slot g's weave in the in-order PE queue
            sc_cur = emit_scores(0)
            for g in range(NSLOT):
                b, ch, tt = g // 64, (g // 16) % 4, g % 16
                sc_next = emit_scores(g + 1) if g + 1 < NSLOT else None
                exp_tile(g, b, ch, tt, sc_cur)
                sc_cur = sc_next
                for u in sched[g]:
                    u()
                # pv lags exp by ~3-5 tiles, shrinking toward the chunk end
                # so accumulation stops by next-chunk slot 1; tt0-2 are held
                # until slot 5 so the pv_read copies clear the banks first
                if tt == 5:
                    for t2 in (0, 1, 2):
                        pv_tile_mm(b, ch, t2)
                elif tt > 5:
                    pv_tile_mm(b, ch, tt - 3)
                elif (b, ch) != (0, 0):
                    pb, pch = (b, ch - 1) if ch else (b - 1, 3)
                    if tt == 0:
                        pv_tile_mm(pb, pch, 13)
                        pv_tile_mm(pb, pch, 14)
                    elif tt == 1:
                        pv_tile_mm(pb, pch, 15)

            for t2 in (13, 14, 15):
                pv_tile_mm(1, 3, t2)
            for u in sched[NSLOT]:
                u()
    nc.compile()
    return nc


def _get_nc():
    if "nc" not in _COMPILED:
        _COMPILED["nc"] = _build()
    return _COMPILED["nc"]


def _hi_lo(x, f8):
    hi = x.astype(f8)
    lo = (x - hi.astype(np.float32)).astype(f8)
    return hi, lo


def _tile_dk(x):
    # [1024, N] -> [128, 8, N] with partition p = d % 128, dk = d // 128
    return np.ascontiguousarray(x.reshape(8, 128, -1).transpose(1, 0, 2))


def _prep_inputs(q, in_w, qkv_bias):
    import ml_dtypes
    f8 = ml_dtypes.float8_e4m3
    qT = q.transpose(2, 0, 1).reshape(D, BS)
    qh, ql = _hi_lo(_tile_dk(qT), f8)
    maps = []
    for c in range(NCORES):
        r = slice(128 * c, 128 * (c + 1))
        wq, wk = in_w[0:D][r], in_w[D:2 * D][r]
        wv = in_w[2 * D:3 * D][r]
        wqk = np.concatenate([wq.T, wk.T], axis=1) * 16.0     # [1024, 256]
        wqkh, wqkl = _hi_lo(_tile_dk(wqk), f8)
        wvh, wvl = _hi_lo(_tile_dk(wv.T * 16.0), f8)
        qkb = np.stack([qkv_bias[0:D][r] / 8.0, qkv_bias[D:2 * D][r]], axis=1)
        maps.append({
            "qh": qh, "ql": ql,
            "wqkh": wqkh, "wqkl": wqkl, "wvh": wvh, "wvl": wvl,
            "w2": None,
            "qkb": np.ascontiguousarray(qkb).astype(np.float32),
        })
    return maps


def kernel(q, k, v, in_w, qkv_bias, out_w, out_b, _trace=False):
    from concourse.bass_utils import run_bass_kernel_spmd

    q = np.asarray(q, dtype=np.float32)
    in_w = np.asarray(in_w, dtype=np.float32)
    qkv_bias = np.asarray(qkv_bias, dtype=np.float32)
    out_w = np.asarray(out_w, dtype=np.float32)
    out_b = np.asarray(out_b, dtype=np.float32)

    nc = _get_nc()
    in_maps = _prep_inputs(q, in_w, qkv_bias)
    for c in range(NCORES):
        r = slice(128 * c, 128 * (c + 1))
        in_maps[c]["w2"] = np.ascontiguousarray(out_w[:, r].T).astype(np.float16)

    res = run_bass_kernel_spmd(
        nc, in_maps, core_ids=list(range(NCORES)), trace=_trace,
    )
    total = np.zeros((BS, D), dtype=np.float32)
    for c in range(NCORES):
        total += res.results[c]["partialT"].astype(np.float32)
    net = total + out_b[None, :] + qkv_bias[2 * D:3 * D] @ out_w.T
    out = net.reshape(B, S, D).astype(np.float32)
    if _trace:
        return out, nc, res
    return out
